# revision 8
# baseline (speedup 1.0000x reference)
"""Multi-head causal self-attention (B=4, T=2048, D=1024, H=16) on 8 TRN2 cores.

Sharding (hardcoded): data-parallel over the 4 batches x tensor-parallel over
head halves. Core c handles batch c//2 and local heads (c%2)*8 .. (c%2)*8+7
for all 2048 positions. The host pre-packs per-core shards: x^T group 0 in
bf16, x^T groups 1-3 in fp8 pair layout, weights in bf16 (fp8 copies are
derived on-chip); it sums the two partial outputs per batch and adds bias bo.

Precision plan (rel-err budget 2e-2; early queries average few keys so fp8
noise doesn't wash out there):
  - group 0 (t<512) Q/K/V projections: bf16, 8x128 contraction chunks
  - groups 1-3 projections: fp8 DoubleRow, 4x256 chunks (2x PE throughput)
  - S^T matmuls: always bf16 (64-contraction row-group pairs)
  - quad 0 attention (queries 0-511): bf16 P/V, per-key-block AV
  - quads 1-3: P written by exp as fp8 (x4 scale folded into the activation
    bias; the softmax denominator rides the same scale so it cancels), V in
    fp8, AV as DoubleRow over key-block PAIRS: one matmul contracts 256 keys
  - out projection: bf16

Per-core schedule: one fluid stream where attention has scheduler priority
(tc.high_priority) and projections fill the tensor engine's stalls, so the
scalar engine (exp) stays saturated while the PE stays dense. DMAs are
ordered by first consumption (x g0 + Wq first) across 4 trigger queues, and
a burst of dummy matmuls/exp warms the PE clock gate + ACT tables during the
initial DMA window.
"""
import numpy as np
import ml_dtypes

import concourse.bass as bass
import concourse.mybir as mybir
import concourse.tile as tile
from concourse import bacc
from concourse.bass_utils import run_bass_kernel_spmd
from concourse.masks import make_upper_triangular

F32 = mybir.dt.float32
BF16 = mybir.dt.bfloat16
F8 = mybir.dt.float8e4
AF = mybir.ActivationFunctionType
DR = mybir.MatmulPerfMode.DoubleRow
BF16NP = ml_dtypes.bfloat16
F8NP = ml_dtypes.float8_e4m3

B, T, D = 4, 2048, 1024
HL = 8              # local heads per core
HP = HL // 2        # local head pairs (two heads share 128 partitions)
DH = 64
PO = D // 128       # contraction chunks over D (bf16 path)
PP = D // 256       # paired contraction chunks (fp8 DoubleRow path)
CD = HL * DH        # 512: local context feature dim
FC = CD // 128      # 4
NB = T // 128       # 16 query/key blocks of 128
NBP = NB // 2       # 8 key-block pairs
QUAD = 4            # query blocks handled together (512 S^T columns)
SCALE = 1.0 / 8.0   # 1/sqrt(DH)
PB = 4.0            # fp8 P pre-scale; cancels in ctx/l
LOG_PB = float(np.log(PB))


def _emit_proj_q0(nc, xt0_sb, mmp, wq_sb, qt_sb):
    """bf16 Q^T projection for group 0 (queries 0-511)."""
    for hp in range(HP):
        ps = mmp.tile([128, 512], F32, tag="mm")
        for po in range(PO):
            nc.tensor.matmul(
                ps, lhsT=wq_sb[:, po, hp * 128:(hp + 1) * 128],
                rhs=xt0_sb[:, po, :],
                start=(po == 0), stop=(po == PO - 1),
            )
        nc.vector.tensor_copy(qt_sb[:, hp, 0:512], ps)


def _emit_proj_kv0(nc, xt0_sb, mmp, wk_sb, wv_sb, kt_sb, vb_sb, v2_sb):
    """bf16 K^T and V projection for group 0 (keys 0-511)."""
    for hp in range(HP):
        ps = mmp.tile([128, 512], F32, tag="mm")
        for po in range(PO):
            nc.tensor.matmul(
                ps, lhsT=wk_sb[:, po, hp * 128:(hp + 1) * 128],
                rhs=xt0_sb[:, po, :],
                start=(po == 0), stop=(po == PO - 1),
            )
        nc.vector.tensor_copy(kt_sb[:, hp, 0:512], ps)
    for tb in range(4):
        kb = tb
        ps = mmp.tile([128, 512], F32, tag="mm")
        for po in range(PO):
            nc.tensor.matmul(
                ps, lhsT=xt0_sb[:, po, tb * 128:(tb + 1) * 128],
                rhs=wv_sb[:, po, :],
                start=(po == 0), stop=(po == PO - 1),
            )
        pr = ps.rearrange("p (h d) -> p h d", h=HL)
        nc.vector.tensor_copy(vb_sb[:, kb, :, 0:64], pr)
        nc.vector.tensor_copy(v2_sb[:, kb // 2, :, kb % 2, 0:64], pr)


def _emit_proj_q8(nc, tw, xt8_sb, mmp, wq8_sb, qt_sb):
    """fp8 DoubleRow Q^T projection for group tw (1..3)."""
    tsl = slice(tw * 512, (tw + 1) * 512)
    for hp in range(HP):
        ps = mmp.tile([128, 512], F32, tag="mm")
        for pp in range(PP):
            nc.tensor.matmul(
                ps, lhsT=wq8_sb[:, pp, :, hp * 128:(hp + 1) * 128],
                rhs=xt8_sb[:, tw - 1, pp, :, :],
                start=(pp == 0), stop=(pp == PP - 1),
                perf_mode=DR,
            )
        nc.vector.tensor_copy(qt_sb[:, hp, tsl], ps)


def _emit_proj_k8_hp(nc, tw, hp, xt8_sb, mmp, wk8_sb, kt_sb):
    """One fp8 DoubleRow K^T projection chain (head pair hp, group tw)."""
    tsl = slice(tw * 512, (tw + 1) * 512)
    ps = mmp.tile([128, 512], F32, tag="mm")
    for pp in range(PP):
        nc.tensor.matmul(
            ps, lhsT=wk8_sb[:, pp, :, hp * 128:(hp + 1) * 128],
            rhs=xt8_sb[:, tw - 1, pp, :, :],
            start=(pp == 0), stop=(pp == PP - 1),
            perf_mode=DR,
        )
    nc.vector.tensor_copy(kt_sb[:, hp, tsl], ps)


def _emit_proj_v8_tb(nc, tw, tb, xt8_sb, mmp, wv8_sb, v2_sb):
    """One fp8 DoubleRow V projection chain (key block tw*4+tb)."""
    kb = tw * 4 + tb
    ps = mmp.tile([128, 512], F32, tag="mm")
    for pp in range(PP):
        nc.tensor.matmul(
            ps, lhsT=xt8_sb[:, tw - 1, pp, :, tb * 128:(tb + 1) * 128],
            rhs=wv8_sb[:, pp, :, :],
            start=(pp == 0), stop=(pp == PP - 1),
            perf_mode=DR,
        )
    nc.vector.tensor_copy(
        v2_sb[:, kb // 2, :, kb % 2, 0:64],
        ps.rearrange("p (h d) -> p h d", h=HL),
    )


def _emit_proj_q8_hp(nc, tw, hp, xt8_sb, mmp, wq8_sb, qt_sb):
    """One fp8 DoubleRow Q^T projection chain (head pair hp, group tw)."""
    tsl = slice(tw * 512, (tw + 1) * 512)
    ps = mmp.tile([128, 512], F32, tag="mm")
    for pp in range(PP):
        nc.tensor.matmul(
            ps, lhsT=wq8_sb[:, pp, :, hp * 128:(hp + 1) * 128],
            rhs=xt8_sb[:, tw - 1, pp, :, :],
            start=(pp == 0), stop=(pp == PP - 1),
            perf_mode=DR,
        )
    nc.vector.tensor_copy(qt_sb[:, hp, tsl], ps)


def _emit_attn_q0(nc, kt_sb, qt_sb, vb_sb, utri01, pbias, stp, ptbp, cxp):
    """Quad 0 attention (queries 0-511): bf16 P/V, per-kb AV, K=128."""
    ctxs = []
    for hp in range(HP):
        ctx = [cxp.tile([65, 512], F32, tag="ctx", name=f"ctx{par}")
               for par in (0, 1)]
        for kb in range(4):
            c0 = kb * 128
            st = stp.tile([128, 2, 512], F32, tag="st")
            for par, lo in ((0, 0), (1, 64)):
                nc.tensor.matmul(
                    st[:, par, c0:512],
                    lhsT=kt_sb[lo:lo + 64, hp, kb * 128:(kb + 1) * 128],
                    rhs=qt_sb[lo:lo + 64, hp, c0:512],
                    start=True, stop=True,
                )
            pt = ptbp.tile([128, 2, 512], BF16, tag="ptb")
            nc.scalar.activation(pt[:, :, c0:512], st[:, :, c0:512],
                                 AF.Exp, scale=SCALE, bias=pbias)
            for par in (0, 1):
                nc.vector.tensor_mul(pt[:, par, c0:c0 + 128],
                                     pt[:, par, c0:c0 + 128], utri01)
            for par in (0, 1):
                nc.tensor.matmul(
                    ctx[par][:, c0:512],
                    lhsT=vb_sb[:, kb, 2 * hp + par, :],
                    rhs=pt[:, par, c0:512],
                    start=(kb == 0), stop=(kb == 3),
                    skip_group_check=True,
                )
        ctxs.append(ctx)
    return ctxs


def _emit_attn_kbps(nc, qb0, kbp_lo, kbp_hi, hp, ctx, kt_sb, qt_sb, v2_sb,
                    utri2, pbias, stp, ptp, last_kbp):
    """fp8 path: S^T -> exp(fp8) -> DoubleRow AV over key-block pairs."""
    for kbp in range(kbp_lo, kbp_hi):
        pt2 = ptp.tile([128, 2, 2, 512], F8, tag="pt")
        c0s = []
        for j in (0, 1):
            kb = 2 * kbp + j
            jd = kb - qb0
            c0 = max(jd, 0) * 128
            c0s.append(c0)
            st = stp.tile([128, 2, 512], F32, tag="st")
            for par, lo in ((0, 0), (1, 64)):
                nc.tensor.matmul(
                    st[:, par, c0:512],
                    lhsT=kt_sb[lo:lo + 64, hp, kb * 128:(kb + 1) * 128],
                    rhs=qt_sb[lo:lo + 64, hp, qb0 * 128 + c0:(qb0 + QUAD) * 128],
                    start=True, stop=True,
                )
            nc.scalar.activation(pt2[:, j, :, c0:512], st[:, :, c0:512],
                                 AF.Exp, scale=SCALE, bias=pbias)
            if jd >= 0:
                # zero future positions in the diagonal key block
                nc.vector.tensor_mul(pt2[:, j, :, c0:c0 + 128],
                                     pt2[:, j, :, c0:c0 + 128], utri2)
        c0min = c0s[0]
        if c0s[1] > c0min:
            # slot 1's first 128 columns were never computed: P must be 0
            nc.gpsimd.memset(pt2[:, 1, :, c0min:c0min + 128], 0.0)
        for par in (0, 1):
            nc.tensor.matmul(
                ctx[par][:, c0min:512],
                lhsT=v2_sb[:, kbp, 2 * hp + par, :, 0:65],
                rhs=pt2[:, :, par, c0min:512],
                start=(kbp == kbp_lo), stop=(kbp == last_kbp),
                perf_mode=DR,
                skip_group_check=True,
            )


def _emit_quad_head_hp(nc, qb0, kbp_hi, hp, kt_sb, qt_sb, v2_sb, utri2,
                       pbias, stp, ptp, cxp, spillp):
    """First kbp_hi key-block pairs of one head pair; ctx spilled to SBUF."""
    ctx = [cxp.tile([65, 512], F32, tag="ctx", name=f"ctx{par}")
           for par in (0, 1)]
    _emit_attn_kbps(nc, qb0, 0, kbp_hi, hp, ctx, kt_sb, qt_sb, v2_sb,
                    utri2, pbias, stp, ptp, kbp_hi - 1)
    sp = []
    for par in (0, 1):
        t = spillp.tile([65, 512], F32, tag="spill",
                        name=f"sp{hp}_{par}")
        nc.vector.tensor_copy(t, ctx[par])
        sp.append(t)
    return sp


def _finish_quad(nc, ctxs, ones65, lvp, ctxt16p, mmp, spills=None):
    """Normalize ctx^T by the l row (row 64) and emit bf16 c16 tiles."""
    ctxt16s = []
    for hp in range(HP):
        ctx = ctxs[hp]
        ctxu = {}
        for par in (0, 1):
            ctxu[par] = lvp.tile([65, 512], BF16, tag="ctxu",
                                 name=f"ctxu{par}")
            if spills is not None:
                nc.vector.tensor_add(ctxu[par], ctx[par], spills[hp][par])
            else:
                nc.vector.tensor_copy(ctxu[par], ctx[par])
        lbinv = {}
        for par in (0, 1):
            ps = mmp.tile([65, 512], F32, tag="mm", name=f"lb{par}")
            nc.tensor.matmul(ps, lhsT=ones65[64:65, :],
                             rhs=ctxu[par][64:65, :], start=True, stop=True)
            lbinv[par] = lvp.tile([65, 512], F32, tag="lbi", name=f"lbi{par}")
            nc.vector.reciprocal_approx_fast(lbinv[par], ps)
        c16 = ctxt16p.tile([128, 512], BF16, tag="c16", name=f"c16_{hp}")
        nc.vector.tensor_mul(c16[0:64, :], ctxu[0][0:64, :], lbinv[0][0:64, :])
        tmp = lvp.tile([64, 512], BF16, tag="ctmp")
        nc.vector.tensor_mul(tmp, ctxu[1][0:64, :], lbinv[1][0:64, :])
        nc.gpsimd.dma_start(c16[64:128, :], tmp)
        ctxt16s.append(c16)
    return ctxt16s


def _emit_quad(nc, qb0, kt_sb, qt_sb, v2_sb, utri2, pbias, ones65,
               stp, ptp, cxp, lvp, ctxt16p, mmp, kbp_lo, spills):
    """fp8 attention for query blocks qb0..qb0+3, all 4 local head pairs."""
    ctxs = []
    for hp in range(HP):
        ctx = [cxp.tile([65, 512], F32, tag="ctx", name=f"ctx{par}")
               for par in (0, 1)]
        _emit_attn_kbps(nc, qb0, kbp_lo, (qb0 + QUAD) // 2, hp, ctx, kt_sb,
                        qt_sb, v2_sb, utri2, pbias, stp, ptp,
                        (qb0 + QUAD) // 2 - 1)
        ctxs.append(ctx)
    return _finish_quad(nc, ctxs, ones65, lvp, ctxt16p, mmp, spills=spills)


def _emit_fill(nc, mmp, utri01, drhs, n):
    """Dummy N=512 matmuls: PE filler to keep the HAM clock gate warm
    through exp-paced attention stretches (throttled PE runs at 1.2 GHz and
    doubles the cost of every real matmul scheduled there)."""
    for _ in range(n):
        ps = mmp.tile([128, 512], F32, tag="mm", name="fill")
        nc.tensor.matmul(ps, lhsT=utri01, rhs=drhs, start=True, stop=True)


def _emit_out_proj_qb(nc, qb0, qloc, ctxt16s, wo_sb, mmp, osbp, out_d):
    """Output projection for query block qb0 + qloc."""
    qb = qb0 + qloc
    for dw in range(2):
        ps = mmp.tile([128, 512], F32, tag="mm")
        for hp in range(HP):
            nc.tensor.matmul(
                ps, lhsT=ctxt16s[hp][:, qloc * 128:(qloc + 1) * 128],
                rhs=wo_sb[:, hp, dw * 512:(dw + 1) * 512],
                start=(hp == 0), stop=(hp == HP - 1),
            )
        osb = osbp.tile([128, 512], BF16, tag="osb")
        nc.vector.tensor_copy(osb, ps)
        nc.sync.dma_start(
            out_d[qb * 128:(qb + 1) * 128, dw * 512:(dw + 1) * 512], osb)


def build_nc():
    nc = bacc.Bacc("TRN2", target_bir_lowering=False)
    x0_d = nc.dram_tensor("x0", [128, PO * 512], BF16, kind="ExternalInput")
    x8_d = nc.dram_tensor("x8", [128, 3 * PO * 512], F8, kind="ExternalInput")
    wq_d = nc.dram_tensor("wq", [128, PO * CD], BF16, kind="ExternalInput")
    wk_d = nc.dram_tensor("wk", [128, PO * CD], BF16, kind="ExternalInput")
    wv_d = nc.dram_tensor("wv", [128, PO * CD], BF16, kind="ExternalInput")
    wo_d = nc.dram_tensor("wo", [128, FC * D], BF16, kind="ExternalInput")
    out_d = nc.dram_tensor("out", [T, D], BF16, kind="ExternalOutput")

    with tile.TileContext(nc) as tc:
        with (
            tc.tile_pool(name="consts", bufs=1) as consts,
            tc.tile_pool(name="wsb", bufs=1) as wsb,
            tc.tile_pool(name="big", bufs=1) as big,
            tc.tile_pool(name="pt", bufs=6) as ptp,
            tc.tile_pool(name="ptb", bufs=4) as ptbp,
            tc.tile_pool(name="lv", bufs=2) as lvp,
            tc.tile_pool(name="ctxt16", bufs=16) as ctxt16p,
            tc.tile_pool(name="osb", bufs=4) as osbp,
            tc.tile_pool(name="spill", bufs=8) as spillp,
            tc.tile_pool(name="dram", bufs=4, space="DRAM") as dramp,
            tc.tile_pool(name="mm", bufs=2, space="PSUM") as mmp,
            tc.tile_pool(name="st", bufs=2, space="PSUM") as stp,
            tc.tile_pool(name="cx", bufs=2, space="PSUM") as cxp,
        ):
            xt0_sb = big.tile([128, PO, 512], BF16, tag="xt0")
            xt8_sb = big.tile([128, 3, PP, 2, 512], F8, tag="xt8")
            kt_sb = big.tile([128, HP, T], BF16, tag="kt")
            qt_sb = big.tile([128, HP, T], BF16, tag="qt")
            vb_sb = big.tile([128, 4, HL, 65], BF16, tag="vb")
            v2_sb = big.tile([128, NBP, HL, 2, 80], F8, tag="v2")
            wq_sb = wsb.tile([128, PO, CD], BF16, tag="wq")
            wk_sb = wsb.tile([128, PO, CD], BF16, tag="wk")
            wv_sb = wsb.tile([128, PO, CD], BF16, tag="wv")
            wo_sb = wsb.tile([128, FC, D], BF16, tag="wo")
            wq8_sb = wsb.tile([128, PP, 2, CD], F8, tag="wq8")
            wk8_sb = wsb.tile([128, PP, 2, CD], F8, tag="wk8")
            wv8_sb = wsb.tile([128, PP, 2, CD], F8, tag="wv8")

            # DMAs first (before any gpsimd memset work so the triggers fire
            # immediately after ucode load), ordered by first consumption
            # across the 3 trigger queues (sync/scalar/gpsimd): x g0 + Wq
            # first, then Wk, Wv, x fp8, Wo, so later transfers don't steal
            # HBM bandwidth from the critical first projections.
            x0_src = x0_d.rearrange("p (a b) -> p a b", a=PO)
            wq_src = wq_d.rearrange("p (a b) -> p a b", a=PO)
            wk_src = wk_d.rearrange("p (a b) -> p a b", a=PO)
            wv_src = wv_d.rearrange("p (a b) -> p a b", a=PO)
            x8_src = x8_d.rearrange("p (g a j b) -> p g a j b", g=3, a=PP, j=2)
            nc.sync.dma_start(xt0_sb[:, 0:4, :], x0_src[:, 0:4, :])
            nc.scalar.dma_start(xt0_sb[:, 4:8, :], x0_src[:, 4:8, :])
            nc.gpsimd.dma_start(wq_sb[:, 0:4, :], wq_src[:, 0:4, :])
            nc.sync.dma_start(wq_sb[:, 4:8, :], wq_src[:, 4:8, :])
            nc.scalar.dma_start(wk_sb[:, 0:4, :], wk_src[:, 0:4, :])
            nc.gpsimd.dma_start(wk_sb[:, 4:8, :], wk_src[:, 4:8, :])
            nc.sync.dma_start(wv_sb[:, 0:4, :], wv_src[:, 0:4, :])
            nc.scalar.dma_start(wv_sb[:, 4:8, :], wv_src[:, 4:8, :])
            nc.gpsimd.dma_start(xt8_sb[:, 0:1], x8_src[:, 0:1])
            nc.sync.dma_start(xt8_sb[:, 1:2], x8_src[:, 1:2])
            nc.scalar.dma_start(xt8_sb[:, 2:3], x8_src[:, 2:3])
            nc.gpsimd.dma_start(wo_sb, wo_d.rearrange("p (a b) -> p a b", a=FC))

            utri01 = consts.tile([128, 128], BF16, tag="utri01")
            make_upper_triangular(nc, utri01, val=1.0, diag=True)
            utri2 = consts.tile([128, 2, 128], BF16, tag="utri2")
            make_upper_triangular(nc, utri2[:, 0, :], val=1.0, diag=True)
            make_upper_triangular(nc, utri2[:, 1, :], val=1.0, diag=True)
            ones65 = consts.tile([65, 65], BF16, tag="ones65")
            nc.vector.memset(ones65, 1.0)
            pbias = consts.tile([128, 1], F32, tag="pbias")
            nc.vector.memset(pbias, LOG_PB)
            nc.vector.memset(vb_sb[:, :, :, 64:65], 1.0)
            nc.vector.memset(v2_sb[:, :, :, :, 64:65], 1.0)

            # warm the PE clock gate + load the ACT exp tables while the
            # input DMAs stream (the first ~3.4us of matmuls run at half
            # clock; the first exp pays a ~2.7us table load)
            warm_ps = mmp.tile([128, 512], F32, tag="mm", name="warm")
            for _ in range(28):
                nc.tensor.matmul(warm_ps[:, 0:128], lhsT=utri01, rhs=utri01,
                                 start=True, stop=True)
            warm_act = consts.tile([64, 128], BF16, tag="warmact")
            nc.scalar.activation(warm_act, utri01[0:64, :], AF.Exp)
            drhs = consts.tile([128, 512], BF16, tag="drhs")
            nc.vector.memset(drhs, 0.5)

            # on-chip bf16 -> fp8 weight copies (pair layout == chunk order)
            nc.vector.tensor_copy(
                wq8_sb, wq_sb.rearrange("p (a j) b -> p a j b", j=2))
            nc.vector.tensor_copy(
                wk8_sb, wk_sb.rearrange("p (a j) b -> p a j b", j=2))
            nc.vector.tensor_copy(
                wv8_sb, wv_sb.rearrange("p (a j) b -> p a j b", j=2))

            _emit_proj_q0(nc, xt0_sb, mmp, wq_sb, qt_sb)
            _emit_proj_kv0(nc, xt0_sb, mmp, wk_sb, wv_sb, kt_sb, vb_sb, v2_sb)
            _emit_proj_q8(nc, 1, xt8_sb, mmp, wq8_sb, qt_sb)
            # Per group: this quad's diagonal tail (its bulk was computed in
            # the previous iteration's head pass), then the next quad's head
            # pass with the NEXT group's K/V (and next-next Q) projection
            # chains interleaved between head-pair rounds, so projections run
            # inside the exp-paced PE idle instead of piling up at the quad
            # boundary (which stalls the exp stream AND lets the PE clock
            # gate re-throttle).
            spills = None
            quad_c16s = []
            for g in range(4):
                if g == 0:
                    with tc.high_priority(offset=2000):
                        ctxs = _emit_attn_q0(nc, kt_sb, qt_sb, vb_sb,
                                             utri01, pbias, stp, ptbp, cxp)
                        ctxt16s = _finish_quad(nc, ctxs, ones65, lvp,
                                               ctxt16p, mmp, spills=None)
                else:
                    with tc.high_priority(offset=2000):
                        ctxt16s = _emit_quad(nc, g * QUAD, kt_sb, qt_sb,
                                             v2_sb, utri2, pbias, ones65,
                                             stp, ptp, cxp, lvp, ctxt16p,
                                             mmp, kbp_lo=2 * g,
                                             spills=spills)
                quad_c16s.append(ctxt16s)
                if g < 3:
                    newspills = []
                    with tc.high_priority(offset=2000):
                        for hp in range(HP):
                            newspills.append(_emit_quad_head_hp(
                                nc, (g + 1) * QUAD, 2 * (g + 1), hp, kt_sb,
                                qt_sb, v2_sb, utri2, pbias, stp, ptp, cxp,
                                spillp))
                            _emit_proj_k8_hp(nc, g + 1, hp, xt8_sb, mmp,
                                             wk8_sb, kt_sb)
                            _emit_proj_v8_tb(nc, g + 1, hp, xt8_sb, mmp,
                                             wv8_sb, v2_sb)
                            if g + 2 <= 3:
                                _emit_proj_q8_hp(nc, g + 2, hp, xt8_sb, mmp,
                                                 wq8_sb, qt_sb)
                    spills = newspills
            # out projections emitted last at normal (low) priority: they
            # become ready as each quad's c16 lands and fill the tensor
            # engine during the exp-paced late attention stretches
            for g in range(4):
                for qloc in range(QUAD):
                    _emit_out_proj_qb(nc, g * QUAD, qloc, quad_c16s[g],
                                      wo_sb, mmp, osbp, out_d)
            # lowest-priority HAM-warming filler: the scheduler drops these
            # into any remaining PE-idle slots
            _emit_fill(nc, mmp, utri01, drhs, 48)

    nc.compile()
    return nc


_CACHE = {}


def _get_nc():
    if "nc" not in _CACHE:
        _CACHE["nc"] = build_nc()
    return _CACHE["nc"]


def _pack_w(w):
    """[128k, N] -> [128, k*N] bf16: partition p holds rows {k*128+p}."""
    k = w.shape[0] // 128
    n = w.shape[1]
    return np.ascontiguousarray(
        w.reshape(k, 128, n).transpose(1, 0, 2).reshape(128, k * n)
    ).astype(BF16NP)


def make_in_maps(x, Wq, Wk, Wv, Wo):
    x = np.asarray(x, np.float32)
    Wq = np.asarray(Wq, np.float32)
    Wk = np.asarray(Wk, np.float32)
    Wv = np.asarray(Wv, np.float32)
    Wo = np.asarray(Wo, np.float32)
    in_maps = []
    for c in range(8):
        b, hh = c // 2, c % 2
        cols = slice(hh * CD, (hh + 1) * CD)
        # x^T in group-major layout: [128, tw, po, 512]; po == (pp, j)
        xt = (x[b].T.reshape(PO, 128, 4, 512).transpose(1, 2, 0, 3))
        x0 = np.ascontiguousarray(xt[:, 0]).reshape(128, PO * 512)
        x8 = np.ascontiguousarray(xt[:, 1:4]).reshape(128, 3 * PO * 512)
        in_maps.append({
            "x0": x0.astype(BF16NP),
            "x8": x8.astype(F8NP),
            "wq": _pack_w(Wq[:, cols]),
            "wk": _pack_w(Wk[:, cols]),
            "wv": _pack_w(Wv[:, cols]),
            "wo": _pack_w(Wo[cols, :]),
        })
    return in_maps


def gather_output(results, bo):
    bo = np.asarray(bo, np.float32)
    out = np.empty((B, T, D), np.float32)
    for b in range(B):
        out[b] = (results[2 * b]["out"].astype(np.float32)
                  + results[2 * b + 1]["out"].astype(np.float32) + bo[None, :])
    return out


def kernel(x, Wq, Wk, Wv, Wo, bo):
    nc = _get_nc()
    in_maps = make_in_maps(x, Wq, Wk, Wv, Wo)
    res = run_bass_kernel_spmd(nc, in_maps, core_ids=list(range(8)))
    return gather_output(res.results, bo)


# revision 9
# speedup vs baseline: 1.0208x; 1.0208x over previous
"""Multi-head causal self-attention (B=4, T=2048, D=1024, H=16) on 8 TRN2 cores.

Sharding (hardcoded): data-parallel over the 4 batches x tensor-parallel over
head halves. Core c handles batch c//2 and local heads (c%2)*8 .. (c%2)*8+7
for all 2048 positions. The host pre-packs per-core shards: x^T group 0 in
bf16, x^T groups 1-3 in fp8 pair layout, weights in bf16 (fp8 copies are
derived on-chip); it sums the two partial outputs per batch and adds bias bo.

Precision plan (rel-err budget 2e-2; early queries average few keys so fp8
noise doesn't wash out there):
  - group 0 (t<512) Q/K/V projections: bf16, 8x128 contraction chunks
  - groups 1-3 projections: fp8 DoubleRow, 4x256 chunks (2x PE throughput)
  - S^T matmuls: always bf16 (64-contraction row-group pairs)
  - quad 0 attention (queries 0-511): bf16 P/V, per-key-block AV
  - quads 1-3: P written by exp as fp8 (x4 scale folded into the activation
    bias; the softmax denominator rides the same scale so it cancels), V in
    fp8, AV as DoubleRow over key-block PAIRS: one matmul contracts 256 keys
  - out projection: bf16

Per-core schedule: one fluid stream where attention has scheduler priority
(tc.high_priority) and projections fill the tensor engine's stalls, so the
scalar engine (exp) stays saturated while the PE stays dense. DMAs are
ordered by first consumption (x g0 + Wq first) across 4 trigger queues, and
a burst of dummy matmuls/exp warms the PE clock gate + ACT tables during the
initial DMA window.
"""
import numpy as np
import ml_dtypes

import concourse.bass as bass
import concourse.mybir as mybir
import concourse.tile as tile
from concourse import bacc
from concourse.bass_utils import run_bass_kernel_spmd
from concourse.masks import make_upper_triangular

F32 = mybir.dt.float32
BF16 = mybir.dt.bfloat16
F8 = mybir.dt.float8e4
AF = mybir.ActivationFunctionType
DR = mybir.MatmulPerfMode.DoubleRow
BF16NP = ml_dtypes.bfloat16
F8NP = ml_dtypes.float8_e4m3

B, T, D = 4, 2048, 1024
HL = 8              # local heads per core
HP = HL // 2        # local head pairs (two heads share 128 partitions)
DH = 64
PO = D // 128       # contraction chunks over D (bf16 path)
PP = D // 256       # paired contraction chunks (fp8 DoubleRow path)
CD = HL * DH        # 512: local context feature dim
FC = CD // 128      # 4
NB = T // 128       # 16 query/key blocks of 128
NBP = NB // 2       # 8 key-block pairs
QUAD = 4            # query blocks handled together (512 S^T columns)
SCALE = 1.0 / 8.0   # 1/sqrt(DH)
PB = 4.0            # fp8 P pre-scale; cancels in ctx/l
LOG_PB = float(np.log(PB))


def _emit_proj_q0(nc, xt0_sb, mmp, wq_sb, qt_sb):
    """bf16 Q^T projection for group 0 (queries 0-511)."""
    for hp in range(HP):
        ps = mmp.tile([128, 512], F32, tag="mm")
        for po in range(PO):
            nc.tensor.matmul(
                ps, lhsT=wq_sb[:, po, hp * 128:(hp + 1) * 128],
                rhs=xt0_sb[:, po, :],
                start=(po == 0), stop=(po == PO - 1),
            )
        nc.vector.tensor_copy(qt_sb[:, hp, 0:512], ps)


def _emit_proj_kv0(nc, xt0_sb, mmp, wk_sb, wv_sb, kt_sb, vb_sb, v2_sb):
    """bf16 K^T and V projection for group 0 (keys 0-511)."""
    for hp in range(HP):
        ps = mmp.tile([128, 512], F32, tag="mm")
        for po in range(PO):
            nc.tensor.matmul(
                ps, lhsT=wk_sb[:, po, hp * 128:(hp + 1) * 128],
                rhs=xt0_sb[:, po, :],
                start=(po == 0), stop=(po == PO - 1),
            )
        nc.vector.tensor_copy(kt_sb[:, hp, 0:512], ps)
    for tb in range(4):
        kb = tb
        ps = mmp.tile([128, 512], F32, tag="mm")
        for po in range(PO):
            nc.tensor.matmul(
                ps, lhsT=xt0_sb[:, po, tb * 128:(tb + 1) * 128],
                rhs=wv_sb[:, po, :],
                start=(po == 0), stop=(po == PO - 1),
            )
        pr = ps.rearrange("p (h d) -> p h d", h=HL)
        nc.vector.tensor_copy(vb_sb[:, kb, :, 0:64], pr)
        nc.vector.tensor_copy(v2_sb[:, kb // 2, :, kb % 2, 0:64], pr)


def _emit_proj_q8(nc, tw, xt8_sb, mmp, wq8_sb, qt_sb):
    """fp8 DoubleRow Q^T projection for group tw (1..3)."""
    tsl = slice(tw * 512, (tw + 1) * 512)
    for hp in range(HP):
        ps = mmp.tile([128, 512], F32, tag="mm")
        for pp in range(PP):
            nc.tensor.matmul(
                ps, lhsT=wq8_sb[:, pp, :, hp * 128:(hp + 1) * 128],
                rhs=xt8_sb[:, tw - 1, pp, :, :],
                start=(pp == 0), stop=(pp == PP - 1),
                perf_mode=DR,
            )
        nc.vector.tensor_copy(qt_sb[:, hp, tsl], ps)


def _emit_proj_k8_hp(nc, tw, hp, xt8_sb, mmp, wk8_sb, kt_sb):
    """One fp8 DoubleRow K^T projection chain (head pair hp, group tw)."""
    tsl = slice(tw * 512, (tw + 1) * 512)
    ps = mmp.tile([128, 512], F32, tag="mm")
    for pp in range(PP):
        nc.tensor.matmul(
            ps, lhsT=wk8_sb[:, pp, :, hp * 128:(hp + 1) * 128],
            rhs=xt8_sb[:, tw - 1, pp, :, :],
            start=(pp == 0), stop=(pp == PP - 1),
            perf_mode=DR,
        )
    nc.vector.tensor_copy(kt_sb[:, hp, tsl], ps)


def _emit_proj_v8_tb(nc, tw, tb, xt8_sb, mmp, wv8_sb, v2_sb):
    """One fp8 DoubleRow V projection chain (key block tw*4+tb)."""
    kb = tw * 4 + tb
    ps = mmp.tile([128, 512], F32, tag="mm")
    for pp in range(PP):
        nc.tensor.matmul(
            ps, lhsT=xt8_sb[:, tw - 1, pp, :, tb * 128:(tb + 1) * 128],
            rhs=wv8_sb[:, pp, :, :],
            start=(pp == 0), stop=(pp == PP - 1),
            perf_mode=DR,
        )
    nc.vector.tensor_copy(
        v2_sb[:, kb // 2, :, kb % 2, 0:64],
        ps.rearrange("p (h d) -> p h d", h=HL),
    )


def _emit_proj_q8_hp(nc, tw, hp, xt8_sb, mmp, wq8_sb, qt_sb):
    """One fp8 DoubleRow Q^T projection chain (head pair hp, group tw)."""
    tsl = slice(tw * 512, (tw + 1) * 512)
    ps = mmp.tile([128, 512], F32, tag="mm")
    for pp in range(PP):
        nc.tensor.matmul(
            ps, lhsT=wq8_sb[:, pp, :, hp * 128:(hp + 1) * 128],
            rhs=xt8_sb[:, tw - 1, pp, :, :],
            start=(pp == 0), stop=(pp == PP - 1),
            perf_mode=DR,
        )
    nc.vector.tensor_copy(qt_sb[:, hp, tsl], ps)


def _emit_attn_q0(nc, kt_sb, qt_sb, vb_sb, utri01, pbias, stp, ptbp, cxp):
    """Quad 0 attention (queries 0-511): bf16 P/V, per-kb AV, K=128."""
    ctxs = []
    for hp in range(HP):
        ctx = [cxp.tile([65, 512], F32, tag="ctx", name=f"ctx{par}")
               for par in (0, 1)]
        for kb in range(4):
            c0 = kb * 128
            st = stp.tile([128, 2, 512], F32, tag="st")
            for par, lo in ((0, 0), (1, 64)):
                nc.tensor.matmul(
                    st[:, par, c0:512],
                    lhsT=kt_sb[lo:lo + 64, hp, kb * 128:(kb + 1) * 128],
                    rhs=qt_sb[lo:lo + 64, hp, c0:512],
                    start=True, stop=True,
                )
            pt = ptbp.tile([128, 2, 512], BF16, tag="ptb")
            nc.scalar.activation(pt[:, :, c0:512], st[:, :, c0:512],
                                 AF.Exp, scale=SCALE, bias=pbias)
            for par in (0, 1):
                nc.gpsimd.tensor_mul(pt[:, par, c0:c0 + 128],
                                     pt[:, par, c0:c0 + 128], utri01)
            for par in (0, 1):
                nc.tensor.matmul(
                    ctx[par][:, c0:512],
                    lhsT=vb_sb[:, kb, 2 * hp + par, :],
                    rhs=pt[:, par, c0:512],
                    start=(kb == 0), stop=(kb == 3),
                    skip_group_check=True,
                )
        ctxs.append(ctx)
    return ctxs


def _emit_attn_kbps(nc, qb0, kbp_lo, kbp_hi, hp, ctx, kt_sb, qt_sb, v2_sb,
                    utri2, pbias, stp, ptp, last_kbp):
    """fp8 path: S^T -> exp(fp8) -> DoubleRow AV over key-block pairs."""
    for kbp in range(kbp_lo, kbp_hi):
        pt2 = ptp.tile([128, 2, 2, 512], F8, tag="pt")
        c0s = []
        for j in (0, 1):
            kb = 2 * kbp + j
            jd = kb - qb0
            c0 = max(jd, 0) * 128
            c0s.append(c0)
            st = stp.tile([128, 2, 512], F32, tag="st")
            for par, lo in ((0, 0), (1, 64)):
                nc.tensor.matmul(
                    st[:, par, c0:512],
                    lhsT=kt_sb[lo:lo + 64, hp, kb * 128:(kb + 1) * 128],
                    rhs=qt_sb[lo:lo + 64, hp, qb0 * 128 + c0:(qb0 + QUAD) * 128],
                    start=True, stop=True,
                )
            nc.scalar.activation(pt2[:, j, :, c0:512], st[:, :, c0:512],
                                 AF.Exp, scale=SCALE, bias=pbias)
            if jd >= 0:
                # zero future positions in the diagonal key block (gpsimd:
                # it is idle and this keeps the exp->AV chain off the DVE)
                nc.gpsimd.tensor_mul(pt2[:, j, :, c0:c0 + 128],
                                     pt2[:, j, :, c0:c0 + 128], utri2)
        c0min = c0s[0]
        if c0s[1] > c0min:
            # slot 1's first 128 columns were never computed: P must be 0
            nc.gpsimd.memset(pt2[:, 1, :, c0min:c0min + 128], 0.0)
        for par in (0, 1):
            nc.tensor.matmul(
                ctx[par][:, c0min:512],
                lhsT=v2_sb[:, kbp, 2 * hp + par, :, 0:65],
                rhs=pt2[:, :, par, c0min:512],
                start=(kbp == kbp_lo), stop=(kbp == last_kbp),
                perf_mode=DR,
                skip_group_check=True,
            )


def _emit_quad_head_hp(nc, qb0, kbp_hi, hp, kt_sb, qt_sb, v2_sb, utri2,
                       pbias, stp, ptp, cxp, spillp):
    """First kbp_hi key-block pairs of one head pair; ctx spilled to SBUF."""
    ctx = [cxp.tile([65, 512], F32, tag="ctx", name=f"ctx{par}")
           for par in (0, 1)]
    _emit_attn_kbps(nc, qb0, 0, kbp_hi, hp, ctx, kt_sb, qt_sb, v2_sb,
                    utri2, pbias, stp, ptp, kbp_hi - 1)
    sp = []
    for par in (0, 1):
        t = spillp.tile([65, 512], F32, tag="spill",
                        name=f"sp{hp}_{par}")
        nc.vector.tensor_copy(t, ctx[par])
        sp.append(t)
    return sp


def _finish_quad(nc, ctxs, ones65, lvp, ctxt16p, mmp, spills=None):
    """Normalize ctx^T by the l row (row 64) and emit bf16 c16 tiles."""
    ctxt16s = []
    for hp in range(HP):
        ctx = ctxs[hp]
        ctxu = {}
        for par in (0, 1):
            ctxu[par] = lvp.tile([65, 512], BF16, tag="ctxu",
                                 name=f"ctxu{par}")
            if spills is not None:
                nc.vector.tensor_add(ctxu[par], ctx[par], spills[hp][par])
            else:
                nc.vector.tensor_copy(ctxu[par], ctx[par])
        lbinv = {}
        for par in (0, 1):
            ps = mmp.tile([65, 512], F32, tag="mm", name=f"lb{par}")
            nc.tensor.matmul(ps, lhsT=ones65[64:65, :],
                             rhs=ctxu[par][64:65, :], start=True, stop=True)
            lbinv[par] = lvp.tile([65, 512], F32, tag="lbi", name=f"lbi{par}")
            nc.vector.reciprocal_approx_fast(lbinv[par], ps)
        c16 = ctxt16p.tile([128, 512], BF16, tag="c16", name=f"c16_{hp}")
        nc.vector.tensor_mul(c16[0:64, :], ctxu[0][0:64, :], lbinv[0][0:64, :])
        tmp = lvp.tile([64, 512], BF16, tag="ctmp")
        nc.vector.tensor_mul(tmp, ctxu[1][0:64, :], lbinv[1][0:64, :])
        nc.gpsimd.dma_start(c16[64:128, :], tmp)
        ctxt16s.append(c16)
    return ctxt16s


def _emit_quad(nc, qb0, kt_sb, qt_sb, v2_sb, utri2, pbias, ones65,
               stp, ptp, cxp, lvp, ctxt16p, mmp, kbp_lo, spills):
    """fp8 attention for query blocks qb0..qb0+3, all 4 local head pairs."""
    ctxs = []
    for hp in range(HP):
        ctx = [cxp.tile([65, 512], F32, tag="ctx", name=f"ctx{par}")
               for par in (0, 1)]
        _emit_attn_kbps(nc, qb0, kbp_lo, (qb0 + QUAD) // 2, hp, ctx, kt_sb,
                        qt_sb, v2_sb, utri2, pbias, stp, ptp,
                        (qb0 + QUAD) // 2 - 1)
        ctxs.append(ctx)
    return _finish_quad(nc, ctxs, ones65, lvp, ctxt16p, mmp, spills=spills)


def _emit_fill(nc, mmp, utri01, drhs, n):
    """Dummy N=512 matmuls: PE filler to keep the HAM clock gate warm
    through exp-paced attention stretches (throttled PE runs at 1.2 GHz and
    doubles the cost of every real matmul scheduled there)."""
    for _ in range(n):
        ps = mmp.tile([128, 512], F32, tag="mm", name="fill")
        nc.tensor.matmul(ps, lhsT=utri01, rhs=drhs, start=True, stop=True)


def _emit_out_proj_qb(nc, qb0, qloc, ctxt16s, wo_sb, mmp, osbp, out_d):
    """Output projection for query block qb0 + qloc."""
    qb = qb0 + qloc
    for dw in range(2):
        ps = mmp.tile([128, 512], F32, tag="mm")
        for hp in range(HP):
            nc.tensor.matmul(
                ps, lhsT=ctxt16s[hp][:, qloc * 128:(qloc + 1) * 128],
                rhs=wo_sb[:, hp, dw * 512:(dw + 1) * 512],
                start=(hp == 0), stop=(hp == HP - 1),
            )
        osb = osbp.tile([128, 512], BF16, tag="osb")
        if dw == 0:
            nc.vector.tensor_copy(osb, ps)
        else:
            nc.scalar.activation(osb, ps, AF.Copy)
        nc.sync.dma_start(
            out_d[qb * 128:(qb + 1) * 128, dw * 512:(dw + 1) * 512], osb)


def build_nc():
    nc = bacc.Bacc("TRN2", target_bir_lowering=False)
    x0_d = nc.dram_tensor("x0", [128, PO * 512], BF16, kind="ExternalInput")
    x8_d = nc.dram_tensor("x8", [128, 3 * PO * 512], F8, kind="ExternalInput")
    wq_d = nc.dram_tensor("wq", [128, PO * CD], BF16, kind="ExternalInput")
    wk_d = nc.dram_tensor("wk", [128, PO * CD], BF16, kind="ExternalInput")
    wv_d = nc.dram_tensor("wv", [128, PO * CD], BF16, kind="ExternalInput")
    wo_d = nc.dram_tensor("wo", [128, FC * D], BF16, kind="ExternalInput")
    out_d = nc.dram_tensor("out", [T, D], BF16, kind="ExternalOutput")

    with tile.TileContext(nc) as tc:
        with (
            tc.tile_pool(name="consts", bufs=1) as consts,
            tc.tile_pool(name="wsb", bufs=1) as wsb,
            tc.tile_pool(name="big", bufs=1) as big,
            tc.tile_pool(name="pt", bufs=6) as ptp,
            tc.tile_pool(name="ptb", bufs=4) as ptbp,
            tc.tile_pool(name="lv", bufs=2) as lvp,
            tc.tile_pool(name="ctxt16", bufs=16) as ctxt16p,
            tc.tile_pool(name="osb", bufs=4) as osbp,
            tc.tile_pool(name="spill", bufs=8) as spillp,
            tc.tile_pool(name="dram", bufs=4, space="DRAM") as dramp,
            tc.tile_pool(name="mm", bufs=2, space="PSUM") as mmp,
            tc.tile_pool(name="st", bufs=2, space="PSUM") as stp,
            tc.tile_pool(name="cx", bufs=2, space="PSUM") as cxp,
        ):
            xt0_sb = big.tile([128, PO, 512], BF16, tag="xt0")
            xt8_sb = big.tile([128, 3, PP, 2, 512], F8, tag="xt8")
            kt_sb = big.tile([128, HP, T], BF16, tag="kt")
            qt_sb = big.tile([128, HP, T], BF16, tag="qt")
            vb_sb = big.tile([128, 4, HL, 65], BF16, tag="vb")
            v2_sb = big.tile([128, NBP, HL, 2, 80], F8, tag="v2")
            wq_sb = wsb.tile([128, PO, CD], BF16, tag="wq")
            wk_sb = wsb.tile([128, PO, CD], BF16, tag="wk")
            wv_sb = wsb.tile([128, PO, CD], BF16, tag="wv")
            wo_sb = wsb.tile([128, FC, D], BF16, tag="wo")
            wq8_sb = wsb.tile([128, PP, 2, CD], F8, tag="wq8")
            wk8_sb = wsb.tile([128, PP, 2, CD], F8, tag="wk8")
            wv8_sb = wsb.tile([128, PP, 2, CD], F8, tag="wv8")

            # DMAs first (before any gpsimd memset work so the triggers fire
            # immediately after ucode load), ordered by first consumption
            # across the 3 trigger queues (sync/scalar/gpsimd): x g0 + Wq
            # first, then Wk, Wv, x fp8, Wo, so later transfers don't steal
            # HBM bandwidth from the critical first projections.
            x0_src = x0_d.rearrange("p (a b) -> p a b", a=PO)
            wq_src = wq_d.rearrange("p (a b) -> p a b", a=PO)
            wk_src = wk_d.rearrange("p (a b) -> p a b", a=PO)
            wv_src = wv_d.rearrange("p (a b) -> p a b", a=PO)
            x8_src = x8_d.rearrange("p (g a j b) -> p g a j b", g=3, a=PP, j=2)
            nc.sync.dma_start(xt0_sb[:, 0:4, :], x0_src[:, 0:4, :])
            nc.scalar.dma_start(xt0_sb[:, 4:8, :], x0_src[:, 4:8, :])
            nc.gpsimd.dma_start(wq_sb[:, 0:4, :], wq_src[:, 0:4, :])
            nc.sync.dma_start(wq_sb[:, 4:8, :], wq_src[:, 4:8, :])
            nc.scalar.dma_start(wk_sb[:, 0:4, :], wk_src[:, 0:4, :])
            nc.gpsimd.dma_start(wk_sb[:, 4:8, :], wk_src[:, 4:8, :])
            nc.sync.dma_start(wv_sb[:, 0:4, :], wv_src[:, 0:4, :])
            nc.scalar.dma_start(wv_sb[:, 4:8, :], wv_src[:, 4:8, :])
            nc.gpsimd.dma_start(xt8_sb[:, 0:1], x8_src[:, 0:1])
            nc.sync.dma_start(xt8_sb[:, 1:2], x8_src[:, 1:2])
            nc.scalar.dma_start(xt8_sb[:, 2:3], x8_src[:, 2:3])
            nc.gpsimd.dma_start(wo_sb, wo_d.rearrange("p (a b) -> p a b", a=FC))

            utri01 = consts.tile([128, 128], BF16, tag="utri01")
            make_upper_triangular(nc, utri01, val=1.0, diag=True)
            utri2 = consts.tile([128, 2, 128], BF16, tag="utri2")
            make_upper_triangular(nc, utri2[:, 0, :], val=1.0, diag=True)
            make_upper_triangular(nc, utri2[:, 1, :], val=1.0, diag=True)
            ones65 = consts.tile([65, 65], BF16, tag="ones65")
            nc.vector.memset(ones65, 1.0)
            pbias = consts.tile([128, 1], F32, tag="pbias")
            nc.vector.memset(pbias, LOG_PB)
            nc.vector.memset(vb_sb[:, :, :, 64:65], 1.0)
            nc.vector.memset(v2_sb[:, :, :, :, 64:65], 1.0)

            # warm the PE clock gate + load the ACT exp tables while the
            # input DMAs stream (the first ~3.4us of matmuls run at half
            # clock; the first exp pays a ~2.7us table load)
            warm_ps = mmp.tile([128, 512], F32, tag="mm", name="warm")
            for _ in range(28):
                nc.tensor.matmul(warm_ps[:, 0:128], lhsT=utri01, rhs=utri01,
                                 start=True, stop=True)
            warm_act = consts.tile([64, 128], BF16, tag="warmact")
            nc.scalar.activation(warm_act, utri01[0:64, :], AF.Exp)
            drhs = consts.tile([128, 512], BF16, tag="drhs")
            nc.vector.memset(drhs, 0.5)

            # on-chip bf16 -> fp8 weight copies (pair layout == chunk order)
            nc.vector.tensor_copy(
                wq8_sb, wq_sb.rearrange("p (a j) b -> p a j b", j=2))
            nc.vector.tensor_copy(
                wk8_sb, wk_sb.rearrange("p (a j) b -> p a j b", j=2))
            nc.vector.tensor_copy(
                wv8_sb, wv_sb.rearrange("p (a j) b -> p a j b", j=2))

            _emit_proj_q0(nc, xt0_sb, mmp, wq_sb, qt_sb)
            _emit_proj_kv0(nc, xt0_sb, mmp, wk_sb, wv_sb, kt_sb, vb_sb, v2_sb)
            _emit_proj_q8(nc, 1, xt8_sb, mmp, wq8_sb, qt_sb)
            # Per group: this quad's diagonal tail (its bulk was computed in
            # the previous iteration's head pass), then the next quad's head
            # pass with the NEXT group's K/V (and next-next Q) projection
            # chains interleaved between head-pair rounds, so projections run
            # inside the exp-paced PE idle instead of piling up at the quad
            # boundary (which stalls the exp stream AND lets the PE clock
            # gate re-throttle).
            spills = None
            quad_c16s = []
            for g in range(4):
                if g == 0:
                    with tc.high_priority(offset=2000):
                        ctxs = _emit_attn_q0(nc, kt_sb, qt_sb, vb_sb,
                                             utri01, pbias, stp, ptbp, cxp)
                        ctxt16s = _finish_quad(nc, ctxs, ones65, lvp,
                                               ctxt16p, mmp, spills=None)
                else:
                    with tc.high_priority(offset=2000):
                        ctxt16s = _emit_quad(nc, g * QUAD, kt_sb, qt_sb,
                                             v2_sb, utri2, pbias, ones65,
                                             stp, ptp, cxp, lvp, ctxt16p,
                                             mmp, kbp_lo=2 * g,
                                             spills=spills)
                quad_c16s.append(ctxt16s)
                if g < 3:
                    newspills = []
                    with tc.high_priority(offset=2000):
                        for hp in range(HP):
                            newspills.append(_emit_quad_head_hp(
                                nc, (g + 1) * QUAD, 2 * (g + 1), hp, kt_sb,
                                qt_sb, v2_sb, utri2, pbias, stp, ptp, cxp,
                                spillp))
                            _emit_proj_k8_hp(nc, g + 1, hp, xt8_sb, mmp,
                                             wk8_sb, kt_sb)
                            _emit_proj_v8_tb(nc, g + 1, hp, xt8_sb, mmp,
                                             wv8_sb, v2_sb)
                            if g + 2 <= 3:
                                _emit_proj_q8_hp(nc, g + 2, hp, xt8_sb, mmp,
                                                 wq8_sb, qt_sb)
                    spills = newspills
            # out projections emitted last at normal (low) priority: they
            # become ready as each quad's c16 lands and fill the tensor
            # engine during the exp-paced late attention stretches
            for g in range(4):
                for qloc in range(QUAD):
                    _emit_out_proj_qb(nc, g * QUAD, qloc, quad_c16s[g],
                                      wo_sb, mmp, osbp, out_d)

    nc.compile()
    return nc


_CACHE = {}


def _get_nc():
    if "nc" not in _CACHE:
        _CACHE["nc"] = build_nc()
    return _CACHE["nc"]


def _pack_w(w):
    """[128k, N] -> [128, k*N] bf16: partition p holds rows {k*128+p}."""
    k = w.shape[0] // 128
    n = w.shape[1]
    return np.ascontiguousarray(
        w.reshape(k, 128, n).transpose(1, 0, 2).reshape(128, k * n)
    ).astype(BF16NP)


def make_in_maps(x, Wq, Wk, Wv, Wo):
    x = np.asarray(x, np.float32)
    Wq = np.asarray(Wq, np.float32)
    Wk = np.asarray(Wk, np.float32)
    Wv = np.asarray(Wv, np.float32)
    Wo = np.asarray(Wo, np.float32)
    in_maps = []
    for c in range(8):
        b, hh = c // 2, c % 2
        cols = slice(hh * CD, (hh + 1) * CD)
        # x^T in group-major layout: [128, tw, po, 512]; po == (pp, j)
        xt = (x[b].T.reshape(PO, 128, 4, 512).transpose(1, 2, 0, 3))
        x0 = np.ascontiguousarray(xt[:, 0]).reshape(128, PO * 512)
        x8 = np.ascontiguousarray(xt[:, 1:4]).reshape(128, 3 * PO * 512)
        in_maps.append({
            "x0": x0.astype(BF16NP),
            "x8": x8.astype(F8NP),
            "wq": _pack_w(Wq[:, cols]),
            "wk": _pack_w(Wk[:, cols]),
            "wv": _pack_w(Wv[:, cols]),
            "wo": _pack_w(Wo[cols, :]),
        })
    return in_maps


def gather_output(results, bo):
    bo = np.asarray(bo, np.float32)
    out = np.empty((B, T, D), np.float32)
    for b in range(B):
        out[b] = (results[2 * b]["out"].astype(np.float32)
                  + results[2 * b + 1]["out"].astype(np.float32) + bo[None, :])
    return out


def kernel(x, Wq, Wk, Wv, Wo, bo):
    nc = _get_nc()
    in_maps = make_in_maps(x, Wq, Wk, Wv, Wo)
    res = run_bass_kernel_spmd(nc, in_maps, core_ids=list(range(8)))
    return gather_output(res.results, bo)
